# revision 3
# baseline (speedup 1.0000x reference)
"""RNN-T Joiner kernel for Trainium2 (Bass/Tile), SPMD over 8 NeuronCores.

Math: logits[b,t,u,v] = (enc@W_enc.T + b_enc + dec@W_dec.T + b_dec) @ W_out.T + b_out
    = A[b,t,v] + C[b,u,v]
where A = enc @ (W_out@W_enc).T  (no bias)
      C = (dec @ W_dec.T) @ W_out.T + (b_enc+b_dec)@W_out.T + b_out

The (B,T,U,512)@(512,500) product in the reference (73.7 GFLOP) collapses by
linearity into two small matmuls plus a broadcast add, leaving the kernel
output-DMA bound (18 MB/core bf16 out, ~57 us at measured ~315 GB/s/core).

Design (per core, 2 batches), v2 — rebuilt from HW microbenchmarks:
- NO Pool/GPSIMD ops in the loop: on real HW each GPSIMD op costs ~2 us
  (the v1 one-hot 'sel' broadcast pipeline measured 129 us standalone vs
  ~20 us modeled). C-row broadcast now uses rank-1 PE matmuls against a
  ones row: crep[p, u, :] = ones[1,128].T @ C[u:u+1, :].
- Matmul rhs base partition must be 0/32/64, so C's 60 rows are staged into
  a [65, 20, 500] SBUF grid (bands at partitions 0/32/64) via a small DRAM
  bounce; rank-1 matmuls slice rows from the band, two rows per PSUM tile,
  then one ACT copy casts [128, <=2, 500] f32 -> bf16 crep group tiles.
- C path reassociated: dec_projT = (dec @ W_dec.T).T via 16 tiny matmuls,
  then C = dec_projT.T @ W_out.T — this removes the 16 big Wcd-fusion
  matmuls and makes C ready earlier (it needs only W_dec.T/dec/W_out).
- The broadcast add A[t,v]+C[u,v] runs as one wide bf16 DVE tensor_add per
  (batch, t-chunk, u-group) (free size 15*500, 2x_1p mode) with A
  free-dim-broadcast; the first group is sliced in 5-u chunks for latency.
- Logits are written bf16 (max rel err ~6e-3 vs the f32 reference); the host
  upcasts on gather. Output DMAs are whole-group [tn, 15, 500] transfers
  (15000 B contiguous DRAM rows) alternating the SP/ACT queues.
- Input DMAs are ordered so the C chain (dec, W_dec.T, W_out) and the first
  A chunk (enc cols 0:128 split out) unblock as early as possible.

Sharding: data-parallel over batch B=16 -> 2 per core, no collectives.
Host-side work is layout only (slice / transpose / reshape) plus the final
bf16->f32 upcast of the gathered output.
"""

import numpy as np

B, T, U, D, V = 16, 300, 30, 512, 500
NCORES = 8
BL = B // NCORES  # batches per core
P = 128
DC = D // P  # 4 contraction chunks

T_CHUNKS = [(0, 128), (128, 128), (256, 44)]
UH = 15  # u-group size (2 groups of 15)
NB = 20  # c_grid band size (rows per base-partition band at 0/32/64)

_CACHE = {}


def _build_program(reps=1):
    from contextlib import ExitStack

    import concourse.bass as bass
    import concourse.tile as tile
    from concourse import bacc, mybir

    f32 = mybir.dt.float32
    f32r = mybir.dt.float32r
    bf16 = mybir.dt.bfloat16

    def r(ap):
        return ap.bitcast(f32r)

    nc = bacc.Bacc("TRN2", target_bir_lowering=False, debug=False)

    enc_t = nc.dram_tensor("enc_t", [D, BL * T], f32, kind="ExternalInput").ap()
    dec_t = nc.dram_tensor("dec_t", [D, BL * U], f32, kind="ExternalInput").ap()
    w_enc = nc.dram_tensor("w_enc", [D, D], f32, kind="ExternalInput").ap()
    w_dec_t = nc.dram_tensor("w_dec_t", [D, D], f32, kind="ExternalInput").ap()
    w_out_t = nc.dram_tensor("w_out_t", [D, V], f32, kind="ExternalInput").ap()
    b_enc_c = nc.dram_tensor("b_enc_c", [P, DC], f32, kind="ExternalInput").ap()
    b_dec_c = nc.dram_tensor("b_dec_c", [P, DC], f32, kind="ExternalInput").ap()
    b_out_r = nc.dram_tensor("b_out_r", [1, V], f32, kind="ExternalInput").ap()
    ones_d = nc.dram_tensor("ones_d", [65, P], f32, kind="ExternalInput").ap()
    c_bounce = nc.dram_tensor("c_bounce", [BL * U, V], f32, kind="Internal").ap()
    out = nc.dram_tensor("out", [BL, T, U, V], bf16, kind="ExternalOutput").ap()

    with tile.TileContext(nc) as tc:
        with ExitStack() as ctx:
            main = ctx.enter_context(tc.tile_pool(name="main", bufs=1))
            ps_small = ctx.enter_context(
                tc.tile_pool(name="ps_small", bufs=2, space="PSUM")
            )
            c_ps = ctx.enter_context(tc.tile_pool(name="c_ps", bufs=1, space="PSUM"))
            crep_ps = ctx.enter_context(
                tc.tile_pool(name="crep_ps", bufs=2, space="PSUM")
            )
            out_pool = ctx.enter_context(tc.tile_pool(name="outp", bufs=3))

            def body():
                # ---- persistent tiles ----
                enc_sb = main.tile([P, DC, BL * T], f32r, name="enc", tag="enc")
                dec_sb = main.tile([P, DC, BL * U], f32r, name="dec", tag="dec")
                wenc_sb = main.tile([P, DC, D], f32r, name="wenc", tag="wenc")
                wdecT_sb = main.tile([P, DC, D], f32r, name="wdecT", tag="wdecT")
                woutT_sb = main.tile([P, DC, V], f32r, name="woutT", tag="woutT")
                benc_sb = main.tile([P, DC], f32r, name="benc", tag="benc")
                bdec_sb = main.tile([P, DC], f32r, name="bdec", tag="bdec")
                bout_sb = main.tile([1, V], f32, name="bout", tag="bout")
                bias_sb = main.tile([1, V], f32r, name="bias", tag="bias")
                ones_sb = main.tile([65, P], f32r, name="ones", tag="ones")
                wceT_sb = [
                    main.tile([P, V], f32r, name=f"wceT{i}", tag=f"wceT{i}")
                    for i in range(DC)
                ]
                dpT_sb = [
                    main.tile([P, BL * U], f32r, name=f"dpT{i}", tag=f"dpT{i}")
                    for i in range(DC)
                ]
                a_sb = [
                    main.tile([P, V], bf16, name=f"a{i}", tag=f"a{i}")
                    for i in range(BL * len(T_CHUNKS))
                ]
                c_sb = main.tile([BL * U, V], f32, name="c", tag="c")
                c_grid = main.tile([65, NB, V], f32r, name="cgrid", tag="cgrid")
                crepg = [
                    main.tile([P, UH, V], bf16, name=f"crepg{i}", tag=f"crepg{i}")
                    for i in range(4)
                ]

                # ---- input DMAs ----
                # sync queue: C-path inputs first, then enc (first chunk split
                # out so A chunk 0 unblocks early)
                nc.sync.dma_start(
                    dec_sb[:], r(dec_t.rearrange("(c p) n -> p c n", c=DC))
                )
                nc.sync.dma_start(
                    wdecT_sb[:], r(w_dec_t.rearrange("(c p) j -> p c j", c=DC))
                )
                enc_r = enc_t.rearrange("(c p) n -> p c n", c=DC)
                nc.sync.dma_start(enc_sb[:, :, 0:P], r(enc_r[:, :, 0:P]))
                nc.sync.dma_start(
                    enc_sb[:, :, P : BL * T], r(enc_r[:, :, P : BL * T])
                )
                # scalar queue: bias/ones (tiny) then w_out, w_enc
                nc.scalar.dma_start(benc_sb[:], r(b_enc_c[:]))
                nc.scalar.dma_start(bdec_sb[:], r(b_dec_c[:]))
                nc.scalar.dma_start(bout_sb[:], b_out_r[:])
                nc.scalar.dma_start(ones_sb[:], r(ones_d[:]))
                nc.scalar.dma_start(
                    woutT_sb[:], r(w_out_t.rearrange("(c p) v -> p c v", c=DC))
                )
                nc.scalar.dma_start(
                    wenc_sb[:], r(w_enc.rearrange("(c p) d -> p c d", c=DC))
                )

                # ---- bias_row = (b_enc + b_dec) @ W_out.T + b_out ----
                ps_b = ps_small.tile([1, V], f32, name="ps", tag="ps")
                for k in range(2 * DC):
                    b_sb = benc_sb if k < DC else bdec_sb
                    nc.tensor.matmul(
                        ps_b[:],
                        b_sb[:, k % DC : k % DC + 1],
                        woutT_sb[:, k % DC, :],
                        start=(k == 0),
                        stop=(k == 2 * DC - 1),
                    )
                nc.vector.tensor_add(bias_sb[:], ps_b[:], bout_sb[:])

                # ---- dec_projT[jc][j, m] = sum_d W_dec[j,d] dec[m,d] ----
                for jc in range(DC):
                    ps = ps_small.tile([P, BL * U], f32, name="ps", tag="ps")
                    for dc in range(DC):
                        nc.tensor.matmul(
                            ps[:],
                            wdecT_sb[:, dc, jc * P : (jc + 1) * P],
                            dec_sb[:, dc, :],
                            start=(dc == 0),
                            stop=(dc == DC - 1),
                        )
                    nc.scalar.copy(dpT_sb[jc][:], ps[:])

                # ---- C[m, v] = dec_proj @ W_out.T + bias_row ----
                ps_c = c_ps.tile([BL * U, V], f32, name="ps_c", tag="ps_c")
                for jc in range(DC):
                    nc.tensor.matmul(
                        ps_c[:],
                        dpT_sb[jc][:],
                        woutT_sb[:, jc, :],
                        start=(jc == 0),
                        stop=False,
                    )
                nc.tensor.matmul(
                    ps_c[:],
                    ones_sb[0:1, : BL * U],
                    bias_sb[:],
                    start=False,
                    stop=True,
                )
                nc.scalar.copy(c_sb[:], ps_c[:])
                # stage C rows into bands at partitions 0/32/64 via DRAM bounce
                nc.sync.dma_start(c_bounce[:, :], c_sb[:])
                cb = c_bounce.rearrange("(b n) v -> b n v", b=3)
                for bb in range(3):
                    nc.sync.dma_start(
                        c_grid[32 * bb : 32 * bb + 1, :, :], r(cb[bb : bb + 1])
                    )

                # ---- Wce fusion with the first A chunk interleaved ----
                def emit_wce_chunk(dc):
                    ps = ps_small.tile([P, V], f32, name="ps", tag="ps")
                    for jc in range(DC):
                        nc.tensor.matmul(
                            ps[:],
                            wenc_sb[:, jc, dc * P : (dc + 1) * P],
                            woutT_sb[:, jc, :],
                            start=(jc == 0),
                            stop=(jc == DC - 1),
                        )
                    nc.scalar.copy(wceT_sb[dc][:], ps[:])

                def emit_a(bl, tci, interleaved=False):
                    t0, tn = T_CHUNKS[tci]
                    n0 = bl * T + t0
                    ps = ps_small.tile([P, V], f32, name="ps", tag="ps")
                    for dc in range(DC):
                        if interleaved:
                            emit_wce_chunk(dc)
                        nc.tensor.matmul(
                            ps[:tn, :],
                            enc_sb[:, dc, n0 : n0 + tn],
                            wceT_sb[dc][:],
                            start=(dc == 0),
                            stop=(dc == DC - 1),
                        )
                    nc.scalar.copy(a_sb[bl * len(T_CHUNKS) + tci][:tn, :], ps[:tn, :])

                # ---- crep: rank-1 broadcast of C rows across 128 partitions ----
                def emit_crep(bl, g):
                    rows = [bl * U + g * UH + j for j in range(UH)]
                    i = 0
                    while i < UH:
                        u = rows[i]
                        bb = u // NB
                        n = 1
                        if i + 1 < UH and rows[i + 1] // NB == bb:
                            n = 2
                        # rows padded to 512 f32 so each stays in one PSUM bank
                        cp = crep_ps.tile([P, 2, 512], f32, name="cps", tag="cps")
                        for k in range(n):
                            nc.tensor.matmul(
                                cp[:, k, 0:V],
                                ones_sb[32 * bb : 32 * bb + 1, :],
                                c_grid[32 * bb : 32 * bb + 1, (u % NB) + k, :],
                                start=True,
                                stop=True,
                            )
                        nc.scalar.copy(
                            crepg[bl * 2 + g][:, i : i + n, :], cp[:, :n, 0:V]
                        )
                        i += n

                # ---- output tiles: ot[t, u, v] = A[t,v] + crep[u][t,v] ----
                qi = [0]

                def emit_out_tile(bl, tci, g, sliced=False):
                    t0, tn = T_CHUNKS[tci]
                    a = a_sb[bl * len(T_CHUNKS) + tci]
                    cg = crepg[bl * 2 + g]
                    ot = out_pool.tile([P, UH, V], bf16, name="ot", tag="ot")
                    ab = a[:tn, :].unsqueeze(1)
                    slices = ((0, 5), (5, 5), (10, 5)) if sliced else ((0, UH),)
                    for s0, sn in slices:
                        nc.vector.tensor_add(
                            ot[:tn, s0 : s0 + sn, :],
                            ab.broadcast_to([tn, sn, V]),
                            cg[:tn, s0 : s0 + sn, :],
                        )
                        q = nc.sync if qi[0] % 2 == 0 else nc.scalar
                        qi[0] += 1
                        q.dma_start(
                            out[bl, t0 : t0 + tn, g * UH + s0 : g * UH + s0 + sn, :],
                            ot[:tn, s0 : s0 + sn, :],
                        )

                emit_a(0, 0, interleaved=True)
                emit_a(0, 1)
                emit_a(0, 2)
                emit_crep(0, 0)
                emit_crep(0, 1)
                for tci in range(len(T_CHUNKS)):
                    emit_a(1, tci)
                emit_out_tile(0, 0, 0, sliced=True)
                emit_out_tile(0, 1, 0)
                emit_out_tile(0, 2, 0)
                emit_crep(1, 0)
                emit_out_tile(0, 0, 1)
                emit_out_tile(0, 1, 1)
                emit_out_tile(0, 2, 1)
                emit_crep(1, 1)
                for g in range(2):
                    for tci in range(len(T_CHUNKS)):
                        emit_out_tile(1, tci, g)

            if reps == 1:
                body()
            else:
                with tc.For_i(
                    0,
                    reps,
                    1,
                    hint_engines=(mybir.EngineType.PE, mybir.EngineType.Activation),
                ):
                    body()

    nc.compile()
    return nc


def _host_prep(inputs):
    """Per-core input maps. Layout-only host work (slice/transpose/reshape)."""
    enc = np.ascontiguousarray(inputs["encoder_out"], dtype=np.float32)
    dec = np.ascontiguousarray(inputs["decoder_out"], dtype=np.float32)
    w_enc = np.ascontiguousarray(inputs["W_enc"], dtype=np.float32)
    w_dec_t = np.ascontiguousarray(inputs["W_dec"].T, dtype=np.float32)
    w_out_t = np.ascontiguousarray(inputs["W_out"].T, dtype=np.float32)
    b_enc_c = np.ascontiguousarray(inputs["b_enc"].reshape(DC, P).T, dtype=np.float32)
    b_dec_c = np.ascontiguousarray(inputs["b_dec"].reshape(DC, P).T, dtype=np.float32)
    b_out_r = np.ascontiguousarray(inputs["b_out"].reshape(1, V), dtype=np.float32)
    ones_np = np.ones((65, P), dtype=np.float32)

    in_maps = []
    for c in range(NCORES):
        b0 = c * BL
        enc_t = np.ascontiguousarray(enc[b0 : b0 + BL].reshape(BL * T, D).T)
        dec_t = np.ascontiguousarray(dec[b0 : b0 + BL].reshape(BL * U, D).T)
        in_maps.append(
            {
                "enc_t": enc_t,
                "dec_t": dec_t,
                "w_enc": w_enc,
                "w_dec_t": w_dec_t,
                "w_out_t": w_out_t,
                "b_enc_c": b_enc_c,
                "b_dec_c": b_dec_c,
                "b_out_r": b_out_r,
                "ones_d": ones_np,
            }
        )
    return in_maps


def get_program(reps=1):
    key = f"nc{reps}"
    if key not in _CACHE:
        _CACHE[key] = _build_program(reps)
    return _CACHE[key]


def kernel(**inputs) -> np.ndarray:
    from concourse.bass_utils import run_bass_kernel_spmd

    nc = get_program()
    in_maps = _host_prep(inputs)
    res = run_bass_kernel_spmd(nc, in_maps, list(range(NCORES)))
    return np.concatenate(
        [np.asarray(r["out"]).astype(np.float32) for r in res.results], axis=0
    )


# revision 5
# speedup vs baseline: 1.0479x; 1.0479x over previous
"""RNN-T Joiner kernel for Trainium2 (Bass/Tile), SPMD over 8 NeuronCores.

Math: logits[b,t,u,v] = (enc@W_enc.T + b_enc + dec@W_dec.T + b_dec) @ W_out.T + b_out
    = A[b,t,v] + C[b,u,v]
where A = enc @ (W_out@W_enc).T  (no bias)
      C = (dec@W_dec.T + b_enc + b_dec) @ W_out.T + b_out

The (B,T,U,512)@(512,500) product in the reference (73.7 GFLOP) collapses by
linearity into two small matmuls plus a broadcast add, leaving the kernel
output-DMA bound (18 MB/core bf16 out, ~57 us at measured ~315 GB/s/core).

Design (per core, 2 batches), v3 — rebuilt from HW microbenchmarks:
- NO Pool/GPSIMD ops: on real HW each GPSIMD op costs ~2 us (the v1 one-hot
  'sel' pipeline measured 129 us standalone vs ~20 us modeled).
- C-row -> 128-partition broadcast via PE matmuls with a stride-0
  (broadcast) identity-column lhsT: crep[p,u,:] = eye[:,u](bcast 128).T @ C.
  Verified on HW; reads C's SBUF tile directly (no staging), any row index.
- C path reassociated: dec_projT = (dec @ W_dec.T).T via 16 tiny matmuls;
  b_enc+b_dec folded into the dec_projT PSUM->SBUF copies as a per-partition
  ACT bias; C = dec_projT.T @ W_out.T + ones.T@b_out (rank-1 inject).
- The broadcast add A[t,v]+C[u,v] runs as one wide bf16 DVE tensor_add per
  (batch, t-chunk, u-group) (free 15*500, 2x_1p mode) with A broadcast on a
  stride-0 free dim; first and last groups are sliced in 5-u chunks to cut
  pipeline head/tail latency.
- crep PSUM rows padded to 512 f32 (PSUM bank alignment), 2 rows per tile,
  3 rotating tiles; copies to bf16 group tiles run on ACT while DVE adds.
- Input DMAs ordered so the C chain (dec, W_dec.T, W_out) unblocks first;
  enc's first 128 columns are a separate DMA so A chunk 0 starts early.
- Output: bf16 logits (rel err ~5e-3), whole-group [tn,15,500] DMAs
  (15000 B contiguous DRAM rows) alternating the SP/ACT queues.

Sharding: data-parallel over batch B=16 -> 2 per core, no collectives.
Host-side work is layout only (slice / transpose / reshape / eye constant)
plus the final bf16->f32 upcast of the gathered output.
"""

import numpy as np

B, T, U, D, V = 16, 300, 30, 512, 500
NCORES = 8
BL = B // NCORES  # batches per core
P = 128
DC = D // P  # 4 contraction chunks

T_CHUNKS = [(0, 128), (128, 128), (256, 44)]
UH = 15  # u-group size (2 groups of 15)

_CACHE = {}


def _build_program(reps=1):
    from contextlib import ExitStack

    import concourse.bass as bass
    import concourse.tile as tile
    from concourse import bacc, mybir

    f32 = mybir.dt.float32
    f32r = mybir.dt.float32r
    bf16 = mybir.dt.bfloat16

    def r(ap):
        return ap.bitcast(f32r)

    nc = bacc.Bacc("TRN2", target_bir_lowering=False, debug=False)

    enc_t = nc.dram_tensor("enc_t", [D, BL * T], f32, kind="ExternalInput").ap()
    dec_t = nc.dram_tensor("dec_t", [D, BL * U], f32, kind="ExternalInput").ap()
    w_enc = nc.dram_tensor("w_enc", [D, D], f32, kind="ExternalInput").ap()
    w_dec_t = nc.dram_tensor("w_dec_t", [D, D], f32, kind="ExternalInput").ap()
    w_out_t = nc.dram_tensor("w_out_t", [D, V], f32, kind="ExternalInput").ap()
    b_enc_c = nc.dram_tensor("b_enc_c", [P, DC], f32, kind="ExternalInput").ap()
    b_dec_c = nc.dram_tensor("b_dec_c", [P, DC], f32, kind="ExternalInput").ap()
    b_out_r = nc.dram_tensor("b_out_r", [1, V], f32, kind="ExternalInput").ap()
    ones_d = nc.dram_tensor("ones_d", [1, P], f32, kind="ExternalInput").ap()
    eye_d = nc.dram_tensor("eye_d", [BL * U, BL * U], f32, kind="ExternalInput").ap()
    out = nc.dram_tensor("out", [BL, T, U, V], bf16, kind="ExternalOutput").ap()

    with tile.TileContext(nc) as tc:
        with ExitStack() as ctx:
            main = ctx.enter_context(tc.tile_pool(name="main", bufs=1))
            ps_w = ctx.enter_context(tc.tile_pool(name="ps_w", bufs=1, space="PSUM"))
            ps_a = ctx.enter_context(tc.tile_pool(name="ps_a", bufs=1, space="PSUM"))
            crep_ps = ctx.enter_context(
                tc.tile_pool(name="crep_ps", bufs=3, space="PSUM")
            )
            out_pool = ctx.enter_context(tc.tile_pool(name="outp", bufs=3))

            def body():
                # ---- persistent tiles ----
                enc_sb = main.tile([P, DC, BL * T], f32r, name="enc", tag="enc")
                dec_sb = main.tile([P, DC, BL * U], f32r, name="dec", tag="dec")
                wenc_sb = main.tile([P, DC, D], f32r, name="wenc", tag="wenc")
                wdecT_sb = main.tile([P, DC, D], f32r, name="wdecT", tag="wdecT")
                woutT_sb = main.tile([P, DC, V], f32r, name="woutT", tag="woutT")
                benc_sb = main.tile([P, DC], f32, name="benc", tag="benc")
                bdec_sb = main.tile([P, DC], f32, name="bdec", tag="bdec")
                bsum_sb = main.tile([P, DC], f32, name="bsum", tag="bsum")
                bout_sb = main.tile([1, V], f32r, name="bout", tag="bout")
                ones_sb = main.tile([1, P], f32r, name="ones", tag="ones")
                eye_sb = main.tile([BL * U, BL * U], f32r, name="eye", tag="eye")
                wceT_sb = [
                    main.tile([P, V], f32r, name=f"wceT{i}", tag=f"wceT{i}")
                    for i in range(DC)
                ]
                dpT_sb = [
                    main.tile([P, BL * U], f32r, name=f"dpT{i}", tag=f"dpT{i}")
                    for i in range(DC)
                ]
                a_sb = [
                    main.tile([P, V], bf16, name=f"a{i}", tag=f"a{i}")
                    for i in range(BL * len(T_CHUNKS))
                ]
                c_sb = main.tile([BL * U, V], f32r, name="c", tag="c")
                crepg = [
                    main.tile([P, UH, V], bf16, name=f"crepg{i}", tag=f"crepg{i}")
                    for i in range(4)
                ]

                # ---- input DMAs (C-path inputs first) ----
                nc.sync.dma_start(bout_sb[:], r(b_out_r[:]))
                nc.sync.dma_start(ones_sb[:], r(ones_d[:]))
                nc.sync.dma_start(eye_sb[:], r(eye_d[:]))
                nc.sync.dma_start(
                    dec_sb[:], r(dec_t.rearrange("(c p) n -> p c n", c=DC))
                )
                nc.sync.dma_start(
                    wdecT_sb[:], r(w_dec_t.rearrange("(c p) j -> p c j", c=DC))
                )
                enc_r = enc_t.rearrange("(c p) n -> p c n", c=DC)
                nc.sync.dma_start(enc_sb[:, :, 0:P], r(enc_r[:, :, 0:P]))
                nc.sync.dma_start(enc_sb[:, :, P : BL * T], r(enc_r[:, :, P : BL * T]))
                nc.scalar.dma_start(benc_sb[:], b_enc_c[:])
                nc.scalar.dma_start(bdec_sb[:], b_dec_c[:])
                nc.scalar.dma_start(
                    woutT_sb[:], r(w_out_t.rearrange("(c p) v -> p c v", c=DC))
                )
                nc.scalar.dma_start(
                    wenc_sb[:], r(w_enc.rearrange("(c p) d -> p c d", c=DC))
                )

                # ---- bsum = b_enc + b_dec (per-partition bias for dec_projT) ----
                nc.vector.tensor_add(bsum_sb[:], benc_sb[:], bdec_sb[:])

                # ---- dec_projT[jc][j, m] = sum_d W_dec[j,d] dec[m,d] + bsum[j] ----
                for jc in range(DC):
                    psd = crep_ps.tile([P, 2, 512], f32, name="cps", tag="cps")
                    ps = psd[:, 0, 0 : BL * U]
                    for dc in range(DC):
                        nc.tensor.matmul(
                            ps,
                            wdecT_sb[:, dc, jc * P : (jc + 1) * P],
                            dec_sb[:, dc, :],
                            start=(dc == 0),
                            stop=(dc == DC - 1),
                        )
                    nc.scalar.activation(
                        dpT_sb[jc][:],
                        ps,
                        mybir.ActivationFunctionType.Identity,
                        bias=bsum_sb[:, jc : jc + 1],
                    )

                # ---- C[m, v] = dec_projT.T @ W_out.T + b_out ----
                ps_c = crep_ps.tile([P, 2, 512], f32, name="cps", tag="cps")
                for jc in range(DC):
                    nc.tensor.matmul(
                        ps_c[0 : BL * U, 0, 0:V],
                        dpT_sb[jc][:],
                        woutT_sb[:, jc, :],
                        start=(jc == 0),
                        stop=False,
                    )
                nc.tensor.matmul(
                    ps_c[0 : BL * U, 0, 0:V],
                    ones_sb[0:1, 0 : BL * U],
                    bout_sb[:],
                    start=False,
                    stop=True,
                )
                nc.scalar.copy(c_sb[:], ps_c[0 : BL * U, 0, 0:V])

                # ---- Wce fusion (copies on DVE; ACT is busy with crep) ----
                def emit_wce_chunk(dc):
                    ps = ps_w.tile([P, V], f32, name="psw", tag="w")
                    for jc in range(DC):
                        nc.tensor.matmul(
                            ps[:],
                            wenc_sb[:, jc, dc * P : (dc + 1) * P],
                            woutT_sb[:, jc, :],
                            start=(jc == 0),
                            stop=(jc == DC - 1),
                        )
                    nc.vector.tensor_copy(wceT_sb[dc][:], ps[:])

                def emit_a(bl, tci, interleaved=False, on_dve=False):
                    t0, tn = T_CHUNKS[tci]
                    n0 = bl * T + t0
                    ps = ps_a.tile([P, V], f32, name="psa", tag="a")
                    for dc in range(DC):
                        if interleaved:
                            emit_wce_chunk(dc)
                        nc.tensor.matmul(
                            ps[:tn, :],
                            enc_sb[:, dc, n0 : n0 + tn],
                            wceT_sb[dc][:],
                            start=(dc == 0),
                            stop=(dc == DC - 1),
                        )
                    dst = a_sb[bl * len(T_CHUNKS) + tci][:tn, :]
                    if on_dve:
                        nc.vector.tensor_copy(dst, ps[:tn, :])
                    else:
                        nc.scalar.copy(dst, ps[:tn, :])

                # ---- crep: broadcast C rows via identity-column matmuls ----
                def emit_crep(bl, g, lo=0, hi=UH):
                    i = lo
                    while i < hi:
                        row = bl * U + g * UH + i
                        n = min(2, hi - i)
                        cp = crep_ps.tile([P, 2, 512], f32, name="cps", tag="cps")
                        for k in range(n):
                            nc.tensor.matmul(
                                cp[:, k, 0:V],
                                eye_sb[:, row + k : row + k + 1].broadcast_to(
                                    [BL * U, P]
                                ),
                                c_sb[:],
                                start=True,
                                stop=True,
                            )
                        nc.scalar.copy(
                            crepg[bl * 2 + g][:, i : i + n, :], cp[:, :n, 0:V]
                        )
                        i += n

                # ---- output tiles: ot[t, u, v] = A[t,v] + crep[u][t,v] ----
                qi = [0]

                def emit_out_tile(bl, tci, g, sliced=False):
                    t0, tn = T_CHUNKS[tci]
                    a = a_sb[bl * len(T_CHUNKS) + tci]
                    cg = crepg[bl * 2 + g]
                    ot = out_pool.tile([P, UH, V], bf16, name="ot", tag="ot")
                    ab = a[:tn, :].unsqueeze(1)
                    slices = ((0, 5), (5, 5), (10, 5)) if sliced else ((0, UH),)
                    for s0, sn in slices:
                        nc.vector.tensor_add(
                            ot[:tn, s0 : s0 + sn, :],
                            ab.broadcast_to([tn, sn, V]),
                            cg[:tn, s0 : s0 + sn, :],
                        )
                        q = nc.sync if qi[0] % 2 == 0 else nc.scalar
                        qi[0] += 1
                        q.dma_start(
                            out[bl, t0 : t0 + tn, g * UH + s0 : g * UH + s0 + sn, :],
                            ot[:tn, s0 : s0 + sn, :],
                        )

                emit_crep(0, 0, 0, 6)
                emit_a(0, 0, interleaved=True, on_dve=True)
                emit_crep(0, 0, 6, UH)
                emit_a(0, 1, on_dve=True)
                emit_out_tile(0, 0, 0, sliced=True)
                emit_crep(0, 1)
                emit_a(0, 2)
                emit_out_tile(0, 1, 0)
                emit_a(1, 0)
                emit_out_tile(0, 2, 0)
                emit_crep(1, 0)
                emit_a(1, 1)
                emit_a(1, 2)
                emit_out_tile(0, 0, 1)
                emit_out_tile(0, 1, 1)
                emit_crep(1, 1)
                emit_out_tile(0, 2, 1)
                emit_out_tile(1, 0, 0)
                emit_out_tile(1, 1, 0)
                emit_out_tile(1, 2, 0)
                emit_out_tile(1, 0, 1)
                emit_out_tile(1, 2, 1)
                emit_out_tile(1, 1, 1, sliced=True)

            if reps == 1:
                body()
            else:
                with tc.For_i(
                    0,
                    reps,
                    1,
                    hint_engines=(mybir.EngineType.PE, mybir.EngineType.Activation),
                ):
                    body()

    nc.compile()
    return nc


def _host_prep(inputs):
    """Per-core input maps. Layout-only host work (slice/transpose/reshape)."""
    enc = np.ascontiguousarray(inputs["encoder_out"], dtype=np.float32)
    dec = np.ascontiguousarray(inputs["decoder_out"], dtype=np.float32)
    w_enc = np.ascontiguousarray(inputs["W_enc"], dtype=np.float32)
    w_dec_t = np.ascontiguousarray(inputs["W_dec"].T, dtype=np.float32)
    w_out_t = np.ascontiguousarray(inputs["W_out"].T, dtype=np.float32)
    b_enc_c = np.ascontiguousarray(inputs["b_enc"].reshape(DC, P).T, dtype=np.float32)
    b_dec_c = np.ascontiguousarray(inputs["b_dec"].reshape(DC, P).T, dtype=np.float32)
    b_out_r = np.ascontiguousarray(inputs["b_out"].reshape(1, V), dtype=np.float32)
    ones_np = np.ones((1, P), dtype=np.float32)
    eye_np = np.eye(BL * U, dtype=np.float32)

    in_maps = []
    for c in range(NCORES):
        b0 = c * BL
        enc_t = np.ascontiguousarray(enc[b0 : b0 + BL].reshape(BL * T, D).T)
        dec_t = np.ascontiguousarray(dec[b0 : b0 + BL].reshape(BL * U, D).T)
        in_maps.append(
            {
                "enc_t": enc_t,
                "dec_t": dec_t,
                "w_enc": w_enc,
                "w_dec_t": w_dec_t,
                "w_out_t": w_out_t,
                "b_enc_c": b_enc_c,
                "b_dec_c": b_dec_c,
                "b_out_r": b_out_r,
                "ones_d": ones_np,
                "eye_d": eye_np,
            }
        )
    return in_maps


def get_program(reps=1):
    key = f"nc{reps}"
    if key not in _CACHE:
        _CACHE[key] = _build_program(reps)
    return _CACHE[key]


def kernel(**inputs) -> np.ndarray:
    from concourse.bass_utils import run_bass_kernel_spmd

    nc = get_program()
    in_maps = _host_prep(inputs)
    res = run_bass_kernel_spmd(nc, in_maps, list(range(NCORES)))
    return np.concatenate(
        [np.asarray(r["out"]).astype(np.float32) for r in res.results], axis=0
    )
